# revision 39
# baseline (speedup 1.0000x reference)
"""Causal self-attention Trainium2 Bass kernel.

Shapes (hardcoded): x [8, 2048, 126] f32, w_attn [126, 378] f32, w_proj [126, 126] f32.
Sharding: data-parallel over batch — one batch element per NeuronCore (8 cores),
no collectives; each core computes its full batch element.

Per-core algorithm (batch b, T=2048, H=6 heads, head_dim=21):
  xT = x_b.T via PE transpose                                     [126, 2048]
  qT/kT = (w_q|w_k)_h.T @ xT  in float32r (near-fp32 precision at
      1 cyc/row), heads on the free dim, replicated on partition
      groups 0/32/64/96 for row-tiling                            [128, 6, 2048]
  v1 = x_b @ w_v (fp32 matmul, bf16 store) with a ones column at
      col 32 per head (fused softmax denominator)                 [128, 6, 16, 33]
  Attention per head, k-tiles in groups of 4 (kc = 4g+r):
     S^T[k, q] chunks: 4 concurrent K=21 matmuls via tile_position
        row-tiling fill the 4 banks of one [128, 2048] PSUM tile
     P^T = exp(S^T / sqrt(21)) — ONE ScalarE activation per chunk,
        PSUM->SBUF bf16 (ScalarE exp is the per-core bottleneck:
        ~123K lane-cycles of causal-masked scores)
     chunk 0 of each group gets a multiplicative 0/1 causal mask (DVE, bf16)
     PV per 512-wide q-block j: PSUM[33, 512] += [v_h|1].T @ P^T —
        row 32 = softmax sum; normalize via gpsimd partition_broadcast
        + DVE reciprocal/multiply, DMA into outF [126, 2048]
  y = outF.T @ w_proj  (K=126, all heads contracted in one matmul per q-tile)
"""

import numpy as np

import concourse.bacc as bacc
import concourse.mybir as mybir
import concourse.tile as tile
from concourse import bass_utils
from concourse.masks import make_identity

B, T, C = 8, 2048, 126
H, D = 6, 21
P = 128
NT = T // P        # 16 k-tiles / q-tiles of 128
NB = T // 512      # 4 q-blocks of 512
SCALE = float(1.0 / np.sqrt(np.float32(D)))
F32 = mybir.dt.float32
F32R = mybir.dt.float32r
BF16 = mybir.dt.bfloat16
EXP = mybir.ActivationFunctionType.Exp
MULT = mybir.AluOpType.mult


def _emit(tc, nc, x, wa_d, wp_d, out):
    with tc.tile_pool(name="persist", bufs=1) as pp:
        # q^T/k^T replicated on 4 partition groups (base 0/32/64/96) so 4
        # k-tiles can run concurrently via tile_position row-tiling; float32r
        # gives near-fp32 score precision at bf16 matmul throughput (N>=256)
        qT = pp.tile([P, H, T], F32R)
        kT = pp.tile([P, H, T], F32R)
        # v per head with a ones column at index 32 (32-aligned so the softmax
        # denominator lands at PSUM partition 32; cols 21..31 stay zero)
        v1 = pp.tile([P, H, NT, 33], BF16)
        outF = pp.tile([C, T], F32)
        wp_sb = pp.tile([C, C], F32)
        # multiplicative causal mask for a 4-ktile group's first chunk:
        # mask01[k, r*512 + c] = 1 if c >= 128*r + k else 0
        mask01 = pp.tile([P, 4, 512], BF16)

        # ---------------- Phase 0: load, transpose, projections ----------------
        with tc.tile_pool(name="ph0", bufs=1) as p0, \
             tc.tile_pool(name="ph0s", bufs=2) as p0s, \
             tc.tile_pool(name="ps0", bufs=2, space="PSUM") as ps0:
            # per-tile DMAs: one big rearranged load is 2048 descriptors of
            # 504B generated serially on the SP sequencer (~20us before any
            # compute can start); 16 per-tile DMAs overlap with the transposes
            # alternate the two DMA dispatch paths (HWDGE via sync, SWDGE via
            # gpsimd) so the 16 loads don't serialize on one dispatcher
            x_sb = p0.tile([P, NT, C], F32)
            for t in range(NT):
                eng = nc.sync if t % 2 == 0 else nc.gpsimd
                eng.dma_start(x_sb[:, t, :], x[t * P:(t + 1) * P, :])
            wa = p0.tile([C, 3 * C], F32)
            nc.sync.dma_start(wa[:], wa_d)
            nc.sync.dma_start(wp_sb[:], wp_d)
            wa_r = p0.tile([C, 3 * C], F32R)
            nc.vector.tensor_copy(wa_r[:], wa[:])
            ident = p0.tile([P, P], F32)
            make_identity(nc, ident[:])
            # keep (1.0) where c - 128*r - k >= 0, else 0
            nc.gpsimd.memset(mask01[:], 1.0)
            nc.gpsimd.affine_select(
                out=mask01[:], in_=mask01[:],
                compare_op=mybir.AluOpType.is_ge, fill=0.0,
                base=0, pattern=[[-P, 4], [1, 512]], channel_multiplier=-1,
            )

            # Startup-critical path, block-major: for each 512-wide block,
            # transpose its 4 x-tiles, project q/k, stage (fp32r), then
            # partition-shift via fat half-width HWDGE DMAs and replicate to
            # partition groups 32/64/96 via the separate SWDGE dispatcher.
            # Everything group 0 needs is ready after nb=0/1. The v projection
            # (only needed later, for PV) is emitted last so its matmuls and
            # copies don't head-of-line-block the DVE/PE queues.
            nc.gpsimd.memset(v1[:], 0.0)
            xT_r = p0.tile([C, T], F32R)
            stgs = {s: p0s.tile([C, T], F32R, tag=f"stg{s}", bufs=1, name=f"stg{s}")
                    for s in (0, 1)}
            for nb in range(NB):
                for t in range(4 * nb, 4 * nb + 4):
                    pst = ps0.tile([C, P], F32, tag="tr")
                    nc.tensor.transpose(pst[:], x_sb[:, t, :], ident[:])
                    nc.vector.tensor_copy(xT_r[:, t * P:(t + 1) * P], pst[:])
                for s in (0, 1):
                    psqk = ps0.tile([C, 512], F32, tag="qk")
                    nc.tensor.matmul(psqk[:], wa_r[:, s * C:(s + 1) * C],
                                     xT_r[:, nb * 512:(nb + 1) * 512],
                                     start=True, stop=True)
                    nc.vector.tensor_copy(stgs[s][:, nb * 512:(nb + 1) * 512],
                                          psqk[:])
                if nb % 2 == 1:
                    half = nb // 2
                    hblk = slice(half * 1024, (half + 1) * 1024)
                    for s, dst in ((0, qT), (1, kT)):
                        for h in range(H):
                            nc.sync.dma_start(dst[0:D, h, hblk],
                                              stgs[s][h * D:(h + 1) * D, hblk])
                    # replicate [0:21] to [32:53], [64:85], [96:117]
                    for nb2 in (nb - 1, nb):
                        blk = slice(nb2 * 512, (nb2 + 1) * 512)
                        for dst in (qT, kT):
                            for r in range(1, 4):
                                nc.gpsimd.dma_start(
                                    dst[32 * r:32 * r + D, :, blk],
                                    dst[0:D, :, blk])

            # v projection (fp32r; v is stored bf16 anyway) with ones column
            for t in range(NT):
                psv = ps0.tile([P, C], F32, tag="v")
                nc.tensor.matmul(psv[:], xT_r[:, t * P:(t + 1) * P],
                                 wa_r[:, 2 * C:3 * C], start=True, stop=True)
                nc.vector.tensor_copy(
                    v1[:, :, t, 0:D],
                    psv[:].rearrange("p (h d) -> p h d", h=H),
                )
            nc.gpsimd.memset(v1[:, :, :, 32:33], 1.0)

        # ---------------- Phase 1: attention per head ----------------
        # k-tiles are processed in groups of 4 (kc = 4g+r) via tile_position
        # row-tiling: 4 concurrent K=21 matmuls on disjoint 32-row PE strips,
        # filling the 4 banks of one [128, 2048] PSUM tile, then ONE exp.
        # PT[g][ci] holds P^T for q-span [512*(g+ci), 512*(g+ci)+512) of all
        # four k-tiles of group g, laid out [128 k, 4 r, 512 q].
        with tc.tile_pool(name="pt", bufs=1) as ptp, \
             tc.tile_pool(name="nrm", bufs=2) as nrm, \
             tc.tile_pool(name="st", bufs=1, space="PSUM") as stp, \
             tc.tile_pool(name="po", bufs=3, space="PSUM") as pop:
            for h in range(H):
                pts = {}

                def pv_block(j, h=h, pts=pts):
                    po = pop.tile([64, 512], F32, tag="po")
                    nkc = 4 * (j + 1)
                    for ki in range(nkc):
                        g, r = ki // 4, ki % 4
                        nc.tensor.matmul(po[0:33, :], v1[:, h, ki, :],
                                         pts[(g, j - g)][:, r, :],
                                         start=(ki == 0), stop=(ki == nkc - 1))
                    ssum = nrm.tile([1, 512], F32, tag="ss")
                    nc.vector.tensor_copy(ssum[:], po[32:33, :])
                    sb21 = nrm.tile([D, 512], F32, tag="sb21")
                    nc.gpsimd.partition_broadcast(sb21[:], ssum[:])
                    rc21 = nrm.tile([D, 512], F32, tag="rc")
                    nc.vector.reciprocal(rc21[:], sb21[:])
                    ot = nrm.tile([D, 512], F32, tag="ot")
                    nc.vector.tensor_tensor(ot[:], po[0:D, :], rc21[:], op=MULT)
                    nc.sync.dma_start(outF[h * D:(h + 1) * D, j * 512:(j + 1) * 512],
                                      ot[:])

                for g in range(NB):
                    for ci in range(NB - g):
                        c = 512 * (g + ci)
                        st = stp.tile([P, 4, 512], F32, tag="st")
                        for r in range(4):
                            kc = 4 * g + r
                            nc.tensor.matmul(st[:, r, :],
                                             kT[32 * r:32 * r + D, h, kc * P:(kc + 1) * P],
                                             qT[32 * r:32 * r + D, h, c:c + 512],
                                             start=True, stop=True,
                                             tile_position=(32 * r, 0))
                        # chunk-0 tiles carry a DVE mask-multiply in their
                        # reuse chain; double-buffer them so the next head's
                        # exp never stalls on a DVE backlog
                        pt = ptp.tile([P, 4, 512], BF16, tag=f"pt{g}_{ci}",
                                      bufs=2 if ci == 0 else 1)
                        pts[(g, ci)] = pt
                        nc.scalar.activation(pt[:], st[:], EXP, bias=0.0, scale=SCALE)
                        if ci == 0:
                            nc.vector.tensor_tensor(pt[:], pt[:], mask01[:], op=MULT)
                    # PV for q-block j=g is ready once chunk (g, 0) exists
                    pv_block(g)

            # ---------- output projection (inside the phase-1 pool scope,
            # sharing a po-pool PSUM bank, so proj tiles for early q-blocks
            # overlap with the last head's attention) ----------
            out_tiled = out.rearrange("(t p) c -> p t c", p=P)
            y_sb = nrm.tile([P, NT, C], F32, tag="y", bufs=1)
            for t in range(NT):
                py = pop.tile([P, C], F32, tag="py", bufs=1)
                nc.tensor.matmul(py[:], outF[:, t * P:(t + 1) * P], wp_sb[:],
                                 start=True, stop=True)
                nc.vector.tensor_copy(y_sb[:, t, :], py[:])
                if t % 4 == 3:
                    nc.sync.dma_start(out_tiled[:, t - 3:t + 1, :],
                                      y_sb[:, t - 3:t + 1, :])


def _build():
    nc = bacc.Bacc("TRN2", target_bir_lowering=False, debug=False, num_devices=B)
    x = nc.dram_tensor("x", [T, C], F32, kind="ExternalInput").ap()
    wa_d = nc.dram_tensor("w_attn", [C, 3 * C], F32, kind="ExternalInput").ap()
    wp_d = nc.dram_tensor("w_proj", [C, C], F32, kind="ExternalInput").ap()
    out = nc.dram_tensor("out", [T, C], F32, kind="ExternalOutput").ap()
    with tile.TileContext(nc) as tc:
        _emit(tc, nc, x, wa_d, wp_d, out)
    nc.compile()
    return nc


_CACHE = {}


def kernel(x, w_attn, w_proj):
    x = np.asarray(x, dtype=np.float32)
    w_attn = np.asarray(w_attn, dtype=np.float32)
    w_proj = np.asarray(w_proj, dtype=np.float32)
    assert x.shape == (B, T, C) and w_attn.shape == (C, 3 * C) and w_proj.shape == (C, C)
    if "nc" not in _CACHE:
        _CACHE["nc"] = _build()
    nc = _CACHE["nc"]
    in_maps = [
        {"x": np.ascontiguousarray(x[b]), "w_attn": w_attn, "w_proj": w_proj}
        for b in range(B)
    ]
    res = bass_utils.run_bass_kernel_spmd(nc, in_maps, core_ids=list(range(B)))
    return np.stack([res.results[b]["out"] for b in range(B)], axis=0)
